# revision 21
# baseline (speedup 1.0000x reference)
"""Causal MHA (B=1, T=4096, D=768, H=12) on 8 TRN2 NeuronCores.

Strategy (v2)
-------------
- Sequence-parallel over T with row-interleaved q-assignment so every core
  runs the *same* program on identically-shaped causal work:
  core c owns q rows {16*(c+8*t)+u}, i.e. 16-row miniblocks strided by 8.
- No collectives (bass collectives run at ~50-60 GB/s with ~10us/step
  latency floors -- an all-gather of K/V would cost 200us+). Each core
  redundantly projects the FULL K^T and V' from the full x, one 512-key
  chunk at a time, software-pipelined with attention over the previous
  chunk. K^T/V' chunks live only in SBUF (bf16) -- no DRAM roundtrip.
- All matmuls in bf16 (PSUM accumulates f32). Scores ~N(0,1), so softmax
  skips the running max: exp(s/8) never overflows. Unnormalized context +
  denominator (ones-column on V') accumulate in SBUF f32 across chunks.
- S^T per head pair runs as two concurrent K=64 row-tiled matmuls
  (tile_position (0,0)/(64,0) auto-derived from base partitions).
- Per-round causal trim: round r covers q cols [64r, 512), one
  r-independent 192-col mask zeroes the sub-diagonal prefix + diagonal.
- exp batched per (pair, kb) across both heads' PSUM banks, with bank
  packing for the small-N rounds to amortize ACT instruction overhead.
"""
import sys

sys.path.insert(0, "/opt/trn_rl_repo")

import numpy as np

import concourse.bass as bass
import concourse.mybir as mybir
import concourse.tile as tile
from concourse.bass_utils import run_bass_kernel_spmd

P = 128
T, D, H, HD = 4096, 768, 12, 64
NC = 8
SQ = T // NC          # 512 q rows per core
CH = 512              # kv chunk (4 key blocks of 128)
DC = D // P           # 6 contraction chunks
NR = 8                # rounds (chunks)
VROW = H * (HD + 1)   # 780: V' row with ones col per head
BF16 = mybir.dt.bfloat16
F32R = mybir.dt.float32r
F32 = mybir.dt.float32


def q_rows(c):
    t = np.arange(32)
    u = np.arange(16)
    return (16 * (c + 8 * t)[:, None] + u[None, :]).reshape(-1)


def make_mask_ext(c):
    """mask_ext[kap, j, m]: for key block kb = 4r+j, q col (abs) 64r+m:
    valid iff 128j + kap <= 16c + 128*(m//16) + (m%16). r-independent."""
    kap = np.arange(128)[:, None, None]
    j = np.arange(4)[None, :, None]
    m = np.arange(192)[None, None, :]
    valid = (128 * j + kap) <= (16 * c + 128 * (m // 16) + (m % 16))
    return valid.astype(np.float32)


def fix_excess_waits(nc):
    """walrus rejects >1 sync wait per instruction; hoist extras onto NoOps."""
    k = 0
    for f in nc.m.functions:
        for bb in f.blocks:
            insts = bb.instructions
            i = 0
            while i < len(insts):
                ins = insts[i]
                si = getattr(ins, "sync_info", None)
                if si is not None and len(si.on_wait) > 1:
                    for w in si.on_wait[:-1]:
                        nop = mybir.InstNoOp(name=f"W-hoist-{k}", ins=[], outs=[])
                        k += 1
                        nop.engine = ins.engine
                        nop.sync_info = mybir.SyncInfo(on_wait=[w], on_update=[])
                        insts.insert(i, nop)
                        i += 1
                    ins.sync_info = mybir.SyncInfo(
                        on_wait=[si.on_wait[-1]], on_update=list(si.on_update))
                i += 1
    return k


def build(fix_waits=True, debug=False):
    nc = bass.Bass()
    xqt = nc.dram_tensor("xqt", [D, SQ], BF16, kind="ExternalInput")
    xt = nc.dram_tensor("xt", [D, T], BF16, kind="ExternalInput")
    wq = nc.dram_tensor("wq", [D, D], BF16, kind="ExternalInput")
    wk = nc.dram_tensor("wk", [D, D], BF16, kind="ExternalInput")
    wv = nc.dram_tensor("wv", [D, D], BF16, kind="ExternalInput")
    wo = nc.dram_tensor("wo", [D, D], BF16, kind="ExternalInput")
    bo = nc.dram_tensor("bo", [P, D], F32, kind="ExternalInput")
    maskx = nc.dram_tensor("maskx", [P, 4, 192], BF16, kind="ExternalInput")
    out = nc.dram_tensor("out", [SQ, D], F32, kind="ExternalOutput")
    if debug:
        dbg_qt = nc.dram_tensor("dbg_qt", [P, DC, SQ], BF16, kind="ExternalOutput")
        dbg_kt = nc.dram_tensor("dbg_kt", [P, DC, CH], BF16, kind="ExternalOutput")
        dbg_vt = nc.dram_tensor("dbg_vt", [P, 4, VROW], BF16, kind="ExternalOutput")
        dbg_pt = nc.dram_tensor("dbg_pt", [P, 2, 4, SQ], BF16, kind="ExternalOutput")
        dbg_ctxA = nc.dram_tensor("dbg_ctxA", [HD + 1, DC, SQ], F32, kind="ExternalOutput")
        dbg_ctxB = nc.dram_tensor("dbg_ctxB", [HD + 1, DC, SQ], F32, kind="ExternalOutput")
        dbg_ctxt = nc.dram_tensor("dbg_ctxt", [P, DC, SQ], BF16, kind="ExternalOutput")

    EXP = mybir.ActivationFunctionType.Exp

    with tile.TileContext(nc) as tc:
        with (
            tc.tile_pool(name="glob", bufs=1) as glob,
            tc.tile_pool(name="xc", bufs=2) as xcp,
            tc.tile_pool(name="ktp", bufs=2) as ktp,
            tc.tile_pool(name="vtp", bufs=2) as vtp,
            tc.tile_pool(name="att", bufs=2) as att,
            tc.tile_pool(name="nrm", bufs=2) as nrm,
            tc.tile_pool(name="ps_s", bufs=2, space="PSUM") as ps_s,
            tc.tile_pool(name="ps_c", bufs=2, space="PSUM") as ps_c,
        ):
            # ---- persistent tiles
            qt = glob.tile([P, DC, SQ], BF16)        # Q^T, head pair i on dc=i
            mask_sb = glob.tile([P, 4, 192], BF16)
            bo_bc = glob.tile([P, D], F32)
            wq_sb = glob.tile([P, DC, D], BF16)
            wk_sb = glob.tile([P, DC, D], BF16)
            wv_sb = glob.tile([P, DC, D], BF16)
            wo_sb = glob.tile([P, DC, D], BF16)
            xq_sb = glob.tile([P, DC, SQ], BF16)
            # unnormalized ctx + denominator row (row 64) per head, f32
            ctxA = glob.tile([HD + 1, DC, SQ], F32)  # even heads 2i
            ctxB = glob.tile([HD + 1, DC, SQ], F32)  # odd heads 2i+1
            ctxt = glob.tile([P, DC, SQ], BF16)      # normalized ctx^T for Wo
            o_sb = glob.tile([P, SQ // P, D], F32)
            ones64 = glob.tile([1, HD], F32R)
            rec_sc = glob.tile([1, H, SQ], F32R)     # per-head 1/den, 1 lane

            nc.sync.dma_start(wq_sb[:], wq.rearrange("(o p) d -> p o d", p=P))
            nc.sync.dma_start(xq_sb[:], xqt.rearrange("(o p) t -> p o t", p=P))
            nc.sync.dma_start(wk_sb[:], wk.rearrange("(o p) d -> p o d", p=P))
            nc.sync.dma_start(wv_sb[:], wv.rearrange("(o p) d -> p o d", p=P))
            nc.sync.dma_start(wo_sb[:], wo.rearrange("(o p) d -> p o d", p=P))
            nc.sync.dma_start(mask_sb[:], maskx[:])
            nc.sync.dma_start(bo_bc[:], bo[:])
            nc.vector.memset(ones64.bitcast(mybir.dt.uint32), 0x3F800000)

            xtv = xt.rearrange("(o p) t -> p o t", p=P)
            xts = [None] * NR
            kts = [None] * NR
            vts = [None] * NR
            for rr in range(2):
                xts[rr] = xcp.tile([P, DC, CH], BF16, tag="xc", name=f"xt{rr}")
                nc.sync.dma_start(xts[rr][:], xtv[:, :, rr * CH:(rr + 1) * CH])

            def proj_units(r):
                """Closures projecting chunk r's K^T and V' (7 units)."""
                units = []

                def k_unit(q2, r=r):
                    if q2 == 0:
                        kts[r] = ktp.tile([P, DC, CH], BF16, tag="kt", name=f"kt{r}")
                    kt = kts[r]
                    xtc = xts[r]
                    pp = ps_s.tile([P, 2, CH], F32, tag="s")
                    for par in range(2):
                        dc = 2 * q2 + par
                        for ko in range(DC):
                            nc.tensor.matmul(
                                pp[:, par, :], wk_sb[:, ko, dc * P:(dc + 1) * P],
                                xtc[:, ko, :], start=(ko == 0), stop=(ko == DC - 1))
                    nc.vector.tensor_copy(kt[:, 2 * q2:2 * q2 + 2, :], pp[:])

                def v_unit(tc4, r=r):
                    if tc4 == 0:
                        vts[r] = vtp.tile([P, 4, VROW], BF16, tag="vt", name=f"vt{r}")
                        v4i = vts[r].rearrange("p f (h c) -> p f h c", c=HD + 1)
                        nc.vector.memset(v4i[:, :, :, HD:HD + 1], 1.0)
                    vt = vts[r]
                    v4 = vt.rearrange("p f (h c) -> p f h c", c=HD + 1)
                    xtc = xts[r]
                    pp = ps_s.tile([P, 2, CH], F32, tag="s")
                    for nh in range(2):
                        for ko in range(DC):
                            nc.tensor.matmul(
                                pp[:, nh, 0:384], xtc[:, ko, tc4 * P:(tc4 + 1) * P],
                                wv_sb[:, ko, nh * 384:(nh + 1) * 384],
                                start=(ko == 0), stop=(ko == DC - 1))
                    nc.vector.tensor_copy(
                        v4[:, tc4, :, 0:HD].rearrange(
                            "p (n h) c -> p n h c", n=2),
                        pp[:, :, 0:384].rearrange("p n (h c) -> p n h c", c=HD))

                for q2 in range(DC // 2):
                    units.append(lambda q2=q2: k_unit(q2))
                for tc4 in range(4):
                    units.append(lambda tc4=tc4: v_unit(tc4))
                return units

            # ---- Q projection: qt[0:64, i] = head 2i, qt[64:128, i] = 2i+1
            for q2 in range(DC // 2):
                pp = ps_s.tile([P, 2, SQ], F32, tag="s")
                for par in range(2):
                    dc = 2 * q2 + par
                    for ko in range(DC):
                        nc.tensor.matmul(
                            pp[:, par, :], wq_sb[:, ko, dc * P:(dc + 1) * P],
                            xq_sb[:, ko, :], start=(ko == 0), stop=(ko == DC - 1))
                nc.vector.tensor_copy(qt[:, 2 * q2:2 * q2 + 2, :], pp[:])

            def norm_units(r):
                """Per-head ctxt normalize for q slice [64r, 64r+64)."""
                s0, s1 = 64 * r, 64 * r + 64
                units = []

                def n_unit(i, hh, ctx):
                    bc = ps_s.tile([P, 2, CH], F32, tag="s")
                    nc.tensor.matmul(
                        bc[0:HD, 0, 0:64], ones64[:],
                        rec_sc[0:1, i + DC * hh, s0:s1],
                        start=True, stop=True)
                    nc.vector.tensor_mul(
                        ctxt[64 * hh:64 * hh + 64, i, s0:s1],
                        ctx[0:HD, i, s0:s1], bc[0:HD, 0, 0:64])

                for i in range(DC):
                    for hh, ctx in ((0, ctxA), (1, ctxB)):
                        units.append(
                            lambda i=i, hh=hh, ctx=ctx: n_unit(i, hh, ctx))
                return units

            def o_unit(m):
                """Project finished t-block m through Wo and DMA it out."""
                op = ps_s.tile([P, 2, CH], F32, tag="s")
                for nh in range(2):
                    for dc in range(DC):
                        nc.tensor.matmul(
                            op[:, nh, 0:384], ctxt[:, dc, m * P:(m + 1) * P],
                            wo_sb[:, dc, nh * 384:(nh + 1) * 384],
                            start=(dc == 0), stop=(dc == DC - 1))
                nc.vector.tensor_add(
                    o_sb[:, m, :].rearrange("p (n c) -> p n c", n=2),
                    op[:, :, 0:384],
                    bo_bc.rearrange("p (n c) -> p n c", n=2))
                nc.sync.dma_start(
                    out.rearrange("(o p) d -> p o d", p=P)[:, m, :],
                    o_sb[:, m, :])

            # chunk 0 projection up front
            for u in proj_units(0):
                u()

            # ---- rounds: attend over chunk r; interleave projection of r+1
            for r in range(NR):
                scope = nc.named_scope(f"round{r}")
                scope.__enter__()
                N = SQ - 64 * r      # live q cols this round
                q0 = 64 * r
                if r + 2 < NR:
                    xts[r + 2] = xcp.tile([P, DC, CH], BF16, tag="xc",
                                        name=f"xt{r + 2}")
                    nc.sync.dma_start(
                        xts[r + 2][:], xtv[:, :, (r + 2) * CH:(r + 3) * CH])
                kt, vt = kts[r], vts[r]
                nrm_q = norm_units(r - 1) if r >= 1 else []
                if debug and r == 0:
                    nc.sync.dma_start(dbg_kt[:], kt[:])
                    nc.sync.dma_start(dbg_vt[:], vt[:])
                nxt = proj_units(r + 1) if r + 1 < NR else []
                # interleave: ~2 norm units per proj unit, O proj after the
                # norm units that complete its t-block
                merged = []
                ni = pi = 0
                while ni < len(nrm_q) or pi < len(nxt):
                    for _ in range(2):
                        if ni < len(nrm_q):
                            merged.append(nrm_q[ni]); ni += 1
                    if ni == len(nrm_q) and r >= 2 and r % 2 == 0 and \
                            not any(u is None for u in merged):
                        merged.append(None)  # placeholder: o_unit
                    if pi < len(nxt):
                        merged.append(nxt[pi]); pi += 1
                if r >= 2 and r % 2 == 0 and not any(u is None for u in merged):
                    merged.append(None)
                mblk = (r - 2) // 2
                nxt = [((lambda m=mblk: o_unit(m)) if u is None else u)
                       for u in merged]

                if N > 256:
                    packs = [[0], [1], [2], [3]]
                elif N > 128:
                    packs = [[0, 1], [2, 3]]
                else:
                    packs = [[0, 1, 2, 3]]
                M = min(192, N)

                npacks = DC * len(packs)
                packs_done = 0
                units_done = 0
                for i in range(DC):
                    pt = att.tile([P, 2, 4, SQ], BF16, tag="pt")
                    cps = ps_c.tile([P, 2, SQ], F32, tag="c")
                    for pk in packs:
                        sps = ps_s.tile([P, 2, CH], F32, tag="s")
                        for idx, j in enumerate(pk):
                            for hh in range(2):
                                nc.tensor.matmul(
                                    sps[0:P, hh, idx * N:(idx + 1) * N],
                                    kt[64 * hh:64 * hh + 64, i, j * P:(j + 1) * P],
                                    qt[64 * hh:64 * hh + 64, i, q0:SQ],
                                    start=True, stop=True)
                        W = len(pk) * N
                        j0 = pk[0]
                        nc.scalar.activation(
                            pt[:, :, j0:j0 + len(pk), 0:N], sps[:, :, 0:W],
                            EXP, scale=0.125)
                        # causal mask for this pack's key blocks
                        for hh, eng in ((0, nc.vector), (1, nc.gpsimd)):
                            eng.tensor_mul(
                                pt[:, hh, j0:j0 + len(pk), 0:M],
                                pt[:, hh, j0:j0 + len(pk), 0:M],
                                mask_sb[:, j0:j0 + len(pk), 0:M])
                        # fill the PE during exp with next-chunk projection
                        # work (at most one unit per pack: the proj psum
                        # shares the ps_c ring with cps)
                        packs_done += 1
                        owed = (packs_done * len(nxt)) // npacks
                        if units_done < owed and units_done < len(nxt):
                            nxt[units_done]()
                            units_done += 1
                        # PV for this pack
                        for j in pk:
                            for hh in range(2):
                                h = 2 * i + hh
                                nc.tensor.matmul(
                                    cps[0:HD + 1, hh, 0:N],
                                    vt[:, j, h * (HD + 1):(h + 1) * (HD + 1)],
                                    pt[:, hh, j, 0:N],
                                    start=(j == 0), stop=(j == 3),
                                    skip_group_check=True)
                    if debug and r == 0 and i == 0:
                        nc.sync.dma_start(dbg_pt[:], pt[:])
                    # drain pair ctx into SBUF accumulators
                    for hh, ctx, eng in ((0, ctxA, nc.vector),
                                               (1, ctxB, nc.vector)):
                        if r == 0:
                            eng.tensor_copy(
                                ctx[:, i, :], cps[0:HD + 1, hh, :])
                        else:
                            eng.tensor_add(
                                ctx[:, i, q0:SQ], ctx[:, i, q0:SQ],
                                cps[0:HD + 1, hh, 0:N])
                # any leftover units (rounds with fewer packs than units)
                for u in nxt[units_done:]:
                    u()
                # q cols [64r, 64r+64) are final now: kick off the
                # reciprocal chain; bc+mult run during the next round.
                s0, s1 = q0, q0 + 64
                recg = nrm.tile([H, 64], F32, tag="rg")
                rec12 = nrm.tile([H, 64], F32R, tag="r12")
                nc.sync.dma_start(recg[0:DC, :], ctxA[HD:HD + 1, :, s0:s1])
                nc.sync.dma_start(recg[DC:H, :], ctxB[HD:HD + 1, :, s0:s1])
                with nc.allow_low_precision(reason="f32r broadcast"):
                    nc.vector.reciprocal(rec12[:], recg[:])
                nc.sync.dma_start(rec_sc[0:1, :, s0:s1], rec12[:])
                scope.__exit__(None, None, None)

            # tail: normalize the last slice, project the final t-block
            for u in norm_units(NR - 1):
                u()
            o_unit(SQ // P - 1)

            if debug:
                nc.sync.dma_start(dbg_qt[:], qt[:])
                nc.sync.dma_start(dbg_ctxA[:], ctxA[:])
                nc.sync.dma_start(dbg_ctxB[:], ctxB[:])

            if debug:
                nc.sync.dma_start(dbg_ctxt[:], ctxt[:])

    if fix_waits:
        fix_excess_waits(nc)
    return nc


_NC_CACHE = None


def _get_nc():
    global _NC_CACHE
    if _NC_CACHE is None:
        _NC_CACHE = build()
    return _NC_CACHE


def _run(inputs, trace=False):
    import ml_dtypes
    bf16 = ml_dtypes.bfloat16

    x = np.asarray(inputs["x"], dtype=np.float32)
    Wq = np.asarray(inputs["Wq"], dtype=np.float32).astype(bf16)
    Wk = np.asarray(inputs["Wk"], dtype=np.float32).astype(bf16)
    Wv = np.asarray(inputs["Wv"], dtype=np.float32).astype(bf16)
    Wo = np.asarray(inputs["Wo"], dtype=np.float32).astype(bf16)
    bo_v = np.ascontiguousarray(
        np.broadcast_to(np.asarray(inputs["bo"], dtype=np.float32).reshape(1, D),
                        (P, D)))
    xf = x.reshape(T, D)
    xt_full = np.ascontiguousarray(xf.T).astype(bf16)

    nc_prog = _get_nc()
    in_maps = []
    for c in range(NC):
        rows = q_rows(c)
        in_maps.append({
            "xqt": np.ascontiguousarray(xf[rows].T).astype(bf16),
            "xt": xt_full,
            "wq": Wq, "wk": Wk, "wv": Wv, "wo": Wo, "bo": bo_v,
            "maskx": make_mask_ext(c).astype(bf16),
        })
    res = run_bass_kernel_spmd(
        nc_prog, in_maps, core_ids=list(range(NC)), trace=trace)
    full = np.empty((T, D), dtype=np.float32)
    for c in range(NC):
        full[q_rows(c)] = res.results[c]["out"]
    return full.reshape(1, T, D), res


def kernel(**inputs) -> np.ndarray:
    out, _ = _run(inputs, trace=False)
    return out


# revision 22
# speedup vs baseline: 1.1656x; 1.1656x over previous
"""Causal MHA (B=1, T=4096, D=768, H=12) on 8 TRN2 NeuronCores.

Strategy (v2)
-------------
- Sequence-parallel over T with row-interleaved q-assignment so every core
  runs the *same* program on identically-shaped causal work:
  core c owns q rows {16*(c+8*t)+u}, i.e. 16-row miniblocks strided by 8.
- No collectives (bass collectives run at ~50-60 GB/s with ~10us/step
  latency floors -- an all-gather of K/V would cost 200us+). Each core
  redundantly projects the FULL K^T and V' from the full x, one 512-key
  chunk at a time, software-pipelined with attention over the previous
  chunk. K^T/V' chunks live only in SBUF (bf16) -- no DRAM roundtrip.
- All matmuls in bf16 (PSUM accumulates f32). Scores ~N(0,1), so softmax
  skips the running max: exp(s/8) never overflows. Unnormalized context +
  denominator (ones-column on V') accumulate in SBUF f32 across chunks.
- S^T per head pair runs as two concurrent K=64 row-tiled matmuls
  (tile_position (0,0)/(64,0) auto-derived from base partitions).
- Per-round causal trim: round r covers q cols [64r, 512), one
  r-independent 192-col mask zeroes the sub-diagonal prefix + diagonal.
- exp batched per (pair, kb) across both heads' PSUM banks, with bank
  packing for the small-N rounds to amortize ACT instruction overhead.
"""
import sys

sys.path.insert(0, "/opt/trn_rl_repo")

import numpy as np

import concourse.bass as bass
import concourse.mybir as mybir
import concourse.tile as tile
from concourse.bass_utils import run_bass_kernel_spmd

P = 128
T, D, H, HD = 4096, 768, 12, 64
NC = 8
SQ = T // NC          # 512 q rows per core
CH = 512              # kv chunk (4 key blocks of 128)
DC = D // P           # 6 contraction chunks
NR = 8                # rounds (chunks)
VROW = H * (HD + 1)   # 780: V' row with ones col per head
BF16 = mybir.dt.bfloat16
F32R = mybir.dt.float32r
F32 = mybir.dt.float32


def q_rows(c):
    t = np.arange(32)
    u = np.arange(16)
    return (16 * (c + 8 * t)[:, None] + u[None, :]).reshape(-1)


def make_mask_ext(c):
    """mask_ext[kap, j, m]: for key block kb = 4r+j, q col (abs) 64r+m:
    valid iff 128j + kap <= 16c + 128*(m//16) + (m%16). r-independent."""
    kap = np.arange(128)[:, None, None]
    j = np.arange(4)[None, :, None]
    m = np.arange(192)[None, None, :]
    valid = (128 * j + kap) <= (16 * c + 128 * (m // 16) + (m % 16))
    return valid.astype(np.float32)


def fix_excess_waits(nc):
    """walrus rejects >1 sync wait per instruction; hoist extras onto NoOps."""
    k = 0
    for f in nc.m.functions:
        for bb in f.blocks:
            insts = bb.instructions
            i = 0
            while i < len(insts):
                ins = insts[i]
                si = getattr(ins, "sync_info", None)
                if si is not None and len(si.on_wait) > 1:
                    for w in si.on_wait[:-1]:
                        nop = mybir.InstNoOp(name=f"W-hoist-{k}", ins=[], outs=[])
                        k += 1
                        nop.engine = ins.engine
                        nop.sync_info = mybir.SyncInfo(on_wait=[w], on_update=[])
                        insts.insert(i, nop)
                        i += 1
                    ins.sync_info = mybir.SyncInfo(
                        on_wait=[si.on_wait[-1]], on_update=list(si.on_update))
                i += 1
    return k


def build(fix_waits=True, debug=False):
    nc = bass.Bass()
    xqt = nc.dram_tensor("xqt", [D, SQ], BF16, kind="ExternalInput")
    xt = nc.dram_tensor("xt", [D, T], BF16, kind="ExternalInput")
    wq = nc.dram_tensor("wq", [D, D], BF16, kind="ExternalInput")
    wk = nc.dram_tensor("wk", [D, D], BF16, kind="ExternalInput")
    wv = nc.dram_tensor("wv", [D, D], BF16, kind="ExternalInput")
    wo = nc.dram_tensor("wo", [D, D], BF16, kind="ExternalInput")
    bo = nc.dram_tensor("bo", [P, D], F32, kind="ExternalInput")
    maskx = nc.dram_tensor("maskx", [P, 4, 192], BF16, kind="ExternalInput")
    out = nc.dram_tensor("out", [SQ, D], F32, kind="ExternalOutput")
    if debug:
        dbg_qt = nc.dram_tensor("dbg_qt", [P, DC, SQ], BF16, kind="ExternalOutput")
        dbg_kt = nc.dram_tensor("dbg_kt", [P, DC, CH], BF16, kind="ExternalOutput")
        dbg_vt = nc.dram_tensor("dbg_vt", [P, 4, VROW], BF16, kind="ExternalOutput")
        dbg_pt = nc.dram_tensor("dbg_pt", [P, 2, 4, SQ], BF16, kind="ExternalOutput")
        dbg_ctxA = nc.dram_tensor("dbg_ctxA", [HD + 1, DC, SQ], F32, kind="ExternalOutput")
        dbg_ctxB = nc.dram_tensor("dbg_ctxB", [HD + 1, DC, SQ], F32, kind="ExternalOutput")
        dbg_ctxt = nc.dram_tensor("dbg_ctxt", [P, DC, SQ], BF16, kind="ExternalOutput")

    EXP = mybir.ActivationFunctionType.Exp

    with tile.TileContext(nc) as tc:
        with (
            tc.tile_pool(name="glob", bufs=1) as glob,
            tc.tile_pool(name="xc", bufs=2) as xcp,
            tc.tile_pool(name="ktp", bufs=2) as ktp,
            tc.tile_pool(name="vtp", bufs=2) as vtp,
            tc.tile_pool(name="att", bufs=2) as att,
            tc.tile_pool(name="nrm", bufs=2) as nrm,
            tc.tile_pool(name="ps_s", bufs=2, space="PSUM") as ps_s,
            tc.tile_pool(name="ps_c", bufs=2, space="PSUM") as ps_c,
        ):
            # ---- persistent tiles
            qt = glob.tile([P, DC, SQ], BF16)        # Q^T, head pair i on dc=i
            mask_sb = glob.tile([P, 4, 192], BF16)
            bo_bc = glob.tile([P, D], F32)
            wq_sb = glob.tile([P, DC, D], BF16)
            wk_sb = glob.tile([P, DC, D], BF16)
            wv_sb = glob.tile([P, DC, D], BF16)
            wo_sb = glob.tile([P, DC, D], BF16)
            xq_sb = glob.tile([P, DC, SQ], BF16)
            # unnormalized ctx + denominator row (row 64) per head, f32
            ctxA = glob.tile([HD + 1, DC, SQ], F32)  # even heads 2i
            ctxB = glob.tile([HD + 1, DC, SQ], F32)  # odd heads 2i+1
            ctxt = glob.tile([P, DC, SQ], BF16)      # normalized ctx^T for Wo
            o_sb = glob.tile([P, SQ // P, D], F32)
            ones64 = glob.tile([1, HD], F32R)
            rec_sc = glob.tile([1, H, SQ], F32R)     # per-head 1/den, 1 lane

            nc.sync.dma_start(wq_sb[:], wq.rearrange("(o p) d -> p o d", p=P))
            nc.sync.dma_start(xq_sb[:], xqt.rearrange("(o p) t -> p o t", p=P))
            nc.sync.dma_start(wk_sb[:], wk.rearrange("(o p) d -> p o d", p=P))
            nc.sync.dma_start(wv_sb[:], wv.rearrange("(o p) d -> p o d", p=P))
            nc.sync.dma_start(wo_sb[:], wo.rearrange("(o p) d -> p o d", p=P))
            nc.sync.dma_start(mask_sb[:], maskx[:])
            nc.sync.dma_start(bo_bc[:], bo[:])
            nc.vector.memset(ones64.bitcast(mybir.dt.uint32), 0x3F800000)

            xtv = xt.rearrange("(o p) t -> p o t", p=P)
            xts = [None] * NR
            kts = [None] * NR
            vts = [None] * NR
            for rr in range(2):
                xts[rr] = xcp.tile([P, DC, CH], BF16, tag="xc", name=f"xt{rr}")
                nc.sync.dma_start(xts[rr][:], xtv[:, :, rr * CH:(rr + 1) * CH])

            def proj_units(r):
                """Closures projecting chunk r's K^T and V' (7 units)."""
                units = []

                def k_unit(q2, r=r):
                    if q2 == 0:
                        kts[r] = ktp.tile([P, DC, CH], BF16, tag="kt", name=f"kt{r}")
                    kt = kts[r]
                    xtc = xts[r]
                    pp = ps_s.tile([P, 2, CH], F32, tag="s")
                    for par in range(2):
                        dc = 2 * q2 + par
                        for ko in range(DC):
                            nc.tensor.matmul(
                                pp[:, par, :], wk_sb[:, ko, dc * P:(dc + 1) * P],
                                xtc[:, ko, :], start=(ko == 0), stop=(ko == DC - 1))
                    nc.vector.tensor_copy(kt[:, 2 * q2:2 * q2 + 2, :], pp[:])

                def v_unit(tc4, r=r):
                    if tc4 == 0:
                        vts[r] = vtp.tile([P, 4, VROW], BF16, tag="vt", name=f"vt{r}")
                        v4i = vts[r].rearrange("p f (h c) -> p f h c", c=HD + 1)
                        nc.vector.memset(v4i[:, :, :, HD:HD + 1], 1.0)
                    vt = vts[r]
                    v4 = vt.rearrange("p f (h c) -> p f h c", c=HD + 1)
                    xtc = xts[r]
                    pp = ps_s.tile([P, 2, CH], F32, tag="s")
                    for nh in range(2):
                        for ko in range(DC):
                            nc.tensor.matmul(
                                pp[:, nh, 0:384], xtc[:, ko, tc4 * P:(tc4 + 1) * P],
                                wv_sb[:, ko, nh * 384:(nh + 1) * 384],
                                start=(ko == 0), stop=(ko == DC - 1))
                    nc.vector.tensor_copy(
                        v4[:, tc4, :, 0:HD].rearrange(
                            "p (n h) c -> p n h c", n=2),
                        pp[:, :, 0:384].rearrange("p n (h c) -> p n h c", c=HD))

                for q2 in range(DC // 2):
                    units.append(lambda q2=q2: k_unit(q2))
                for tc4 in range(4):
                    units.append(lambda tc4=tc4: v_unit(tc4))
                return units

            # ---- Q projection: qt[0:64, i] = head 2i, qt[64:128, i] = 2i+1
            for q2 in range(DC // 2):
                pp = ps_s.tile([P, 2, SQ], F32, tag="s")
                for par in range(2):
                    dc = 2 * q2 + par
                    for ko in range(DC):
                        nc.tensor.matmul(
                            pp[:, par, :], wq_sb[:, ko, dc * P:(dc + 1) * P],
                            xq_sb[:, ko, :], start=(ko == 0), stop=(ko == DC - 1))
                nc.vector.tensor_copy(qt[:, 2 * q2:2 * q2 + 2, :], pp[:])

            def o_unit(m):
                """Project finished t-block m through Wo and DMA it out."""
                op = ps_s.tile([P, 2, CH], F32, tag="s")
                for nh in range(2):
                    for dc in range(DC):
                        nc.tensor.matmul(
                            op[:, nh, 0:384], ctxt[:, dc, m * P:(m + 1) * P],
                            wo_sb[:, dc, nh * 384:(nh + 1) * 384],
                            start=(dc == 0), stop=(dc == DC - 1))
                nc.vector.tensor_add(
                    o_sb[:, m, :].rearrange("p (n c) -> p n c", n=2),
                    op[:, :, 0:384],
                    bo_bc.rearrange("p (n c) -> p n c", n=2))
                nc.sync.dma_start(
                    out.rearrange("(o p) d -> p o d", p=P)[:, m, :],
                    o_sb[:, m, :])

            # chunk 0 projection up front
            for u in proj_units(0):
                u()

            # ---- rounds: attend over chunk r; interleave projection of r+1
            for r in range(NR):
                scope = nc.named_scope(f"round{r}")
                scope.__enter__()
                N = SQ - 64 * r      # live q cols this round
                q0 = 64 * r
                if r + 2 < NR:
                    xts[r + 2] = xcp.tile([P, DC, CH], BF16, tag="xc",
                                        name=f"xt{r + 2}")
                    nc.sync.dma_start(
                        xts[r + 2][:], xtv[:, :, (r + 2) * CH:(r + 3) * CH])
                kt, vt = kts[r], vts[r]
                if debug and r == 0:
                    nc.sync.dma_start(dbg_kt[:], kt[:])
                    nc.sync.dma_start(dbg_vt[:], vt[:])
                nxt = proj_units(r + 1) if r + 1 < NR else []

                if N > 256:
                    packs = [[0], [1], [2], [3]]
                elif N > 128:
                    packs = [[0, 1], [2, 3]]
                else:
                    packs = [[0, 1, 2, 3]]
                M = min(192, N)

                npacks = DC * len(packs)
                packs_done = 0
                units_done = 0
                for i in range(DC):
                    pt = att.tile([P, 2, 4, SQ], BF16, tag="pt")
                    cps = ps_c.tile([P, 2, SQ], F32, tag="c")
                    for pk in packs:
                        sps = ps_s.tile([P, 2, CH], F32, tag="s")
                        for idx, j in enumerate(pk):
                            for hh in range(2):
                                nc.tensor.matmul(
                                    sps[0:P, hh, idx * N:(idx + 1) * N],
                                    kt[64 * hh:64 * hh + 64, i, j * P:(j + 1) * P],
                                    qt[64 * hh:64 * hh + 64, i, q0:SQ],
                                    start=True, stop=True)
                        W = len(pk) * N
                        j0 = pk[0]
                        nc.scalar.activation(
                            pt[:, :, j0:j0 + len(pk), 0:N], sps[:, :, 0:W],
                            EXP, scale=0.125)
                        # causal mask for this pack's key blocks
                        for hh, eng in ((0, nc.vector), (1, nc.gpsimd)):
                            eng.tensor_mul(
                                pt[:, hh, j0:j0 + len(pk), 0:M],
                                pt[:, hh, j0:j0 + len(pk), 0:M],
                                mask_sb[:, j0:j0 + len(pk), 0:M])
                        # fill the PE during exp with next-chunk projection
                        # work (at most one unit per pack: the proj psum
                        # shares the ps_c ring with cps)
                        packs_done += 1
                        owed = (packs_done * len(nxt)) // npacks
                        if units_done < owed and units_done < len(nxt):
                            nxt[units_done]()
                            units_done += 1
                        # PV for this pack
                        for j in pk:
                            for hh in range(2):
                                h = 2 * i + hh
                                nc.tensor.matmul(
                                    cps[0:HD + 1, hh, 0:N],
                                    vt[:, j, h * (HD + 1):(h + 1) * (HD + 1)],
                                    pt[:, hh, j, 0:N],
                                    start=(j == 0), stop=(j == 3),
                                    skip_group_check=True)
                    if debug and r == 0 and i == 0:
                        nc.sync.dma_start(dbg_pt[:], pt[:])
                    # drain pair ctx into SBUF accumulators
                    for hh, ctx, eng in ((0, ctxA, nc.vector),
                                               (1, ctxB, nc.vector)):
                        if r == 0:
                            eng.tensor_copy(
                                ctx[:, i, :], cps[0:HD + 1, hh, :])
                        else:
                            eng.tensor_add(
                                ctx[:, i, q0:SQ], ctx[:, i, q0:SQ],
                                cps[0:HD + 1, hh, 0:N])
                # any leftover units (rounds with fewer packs than units)
                for u in nxt[units_done:]:
                    u()
                scope.__exit__(None, None, None)

            # ---- tail: normalize all heads, then project + emit blocks
            recg = nrm.tile([H, SQ], F32, tag="rg")
            rec12 = nrm.tile([H, SQ], F32R, tag="r12")
            nc.sync.dma_start(recg[0:DC, :], ctxA[HD:HD + 1, :, :])
            nc.sync.dma_start(recg[DC:H, :], ctxB[HD:HD + 1, :, :])
            with nc.allow_low_precision(reason="f32r broadcast"):
                nc.vector.reciprocal(rec12[:], recg[:])
            nc.sync.dma_start(rec_sc[:], rec12[:])
            for i in range(DC):
                for hh, ctx in ((0, ctxA), (1, ctxB)):
                    bc = ps_c.tile([HD, 2, SQ], F32, tag="c")
                    nc.tensor.matmul(
                        bc[:, 0, :], ones64[:], rec_sc[0:1, i + DC * hh, :],
                        start=True, stop=True)
                    nc.vector.tensor_mul(
                        ctxt[64 * hh:64 * hh + 64, i, :],
                        ctx[0:HD, i, :], bc[:, 0, :])
            for m in range(SQ // P):
                o_unit(m)

            if debug:
                nc.sync.dma_start(dbg_qt[:], qt[:])
                nc.sync.dma_start(dbg_ctxA[:], ctxA[:])
                nc.sync.dma_start(dbg_ctxB[:], ctxB[:])

            if debug:
                nc.sync.dma_start(dbg_ctxt[:], ctxt[:])

    if fix_waits:
        fix_excess_waits(nc)
    return nc


_NC_CACHE = None


def _get_nc():
    global _NC_CACHE
    if _NC_CACHE is None:
        _NC_CACHE = build()
    return _NC_CACHE


def _run(inputs, trace=False):
    import ml_dtypes
    bf16 = ml_dtypes.bfloat16

    x = np.asarray(inputs["x"], dtype=np.float32)
    Wq = np.asarray(inputs["Wq"], dtype=np.float32).astype(bf16)
    Wk = np.asarray(inputs["Wk"], dtype=np.float32).astype(bf16)
    Wv = np.asarray(inputs["Wv"], dtype=np.float32).astype(bf16)
    Wo = np.asarray(inputs["Wo"], dtype=np.float32).astype(bf16)
    bo_v = np.ascontiguousarray(
        np.broadcast_to(np.asarray(inputs["bo"], dtype=np.float32).reshape(1, D),
                        (P, D)))
    xf = x.reshape(T, D)
    xt_full = np.ascontiguousarray(xf.T).astype(bf16)

    nc_prog = _get_nc()
    in_maps = []
    for c in range(NC):
        rows = q_rows(c)
        in_maps.append({
            "xqt": np.ascontiguousarray(xf[rows].T).astype(bf16),
            "xt": xt_full,
            "wq": Wq, "wk": Wk, "wv": Wv, "wo": Wo, "bo": bo_v,
            "maskx": make_mask_ext(c).astype(bf16),
        })
    res = run_bass_kernel_spmd(
        nc_prog, in_maps, core_ids=list(range(NC)), trace=trace)
    full = np.empty((T, D), dtype=np.float32)
    for c in range(NC):
        full[q_rows(c)] = res.results[c]["out"]
    return full.reshape(1, T, D), res


def kernel(**inputs) -> np.ndarray:
    out, _ = _run(inputs, trace=False)
    return out


# revision 23
# speedup vs baseline: 1.2453x; 1.0684x over previous
"""Causal MHA (B=1, T=4096, D=768, H=12) on 8 TRN2 NeuronCores.

Strategy (v2)
-------------
- Sequence-parallel over T with row-interleaved q-assignment so every core
  runs the *same* program on identically-shaped causal work:
  core c owns q rows {16*(c+8*t)+u}, i.e. 16-row miniblocks strided by 8.
- No collectives (bass collectives run at ~50-60 GB/s with ~10us/step
  latency floors -- an all-gather of K/V would cost 200us+). Each core
  redundantly projects the FULL K^T and V' from the full x, one 512-key
  chunk at a time, software-pipelined with attention over the previous
  chunk. K^T/V' chunks live only in SBUF (bf16) -- no DRAM roundtrip.
- All matmuls in bf16 (PSUM accumulates f32). Scores ~N(0,1), so softmax
  skips the running max: exp(s/8) never overflows. Unnormalized context +
  denominator (ones-column on V') accumulate in SBUF f32 across chunks.
- S^T per head pair runs as two concurrent K=64 row-tiled matmuls
  (tile_position (0,0)/(64,0) auto-derived from base partitions).
- Per-round causal trim: round r covers q cols [64r, 512), one
  r-independent 192-col mask zeroes the sub-diagonal prefix + diagonal.
- exp batched per (pair, kb) across both heads' PSUM banks, with bank
  packing for the small-N rounds to amortize ACT instruction overhead.
"""
import sys

sys.path.insert(0, "/opt/trn_rl_repo")

import numpy as np

import concourse.bass as bass
import concourse.mybir as mybir
import concourse.tile as tile
from concourse.bass_utils import run_bass_kernel_spmd

P = 128
T, D, H, HD = 4096, 768, 12, 64
NC = 8
SQ = T // NC          # 512 q rows per core
CH = 512              # kv chunk (4 key blocks of 128)
DC = D // P           # 6 contraction chunks
NR = 8                # rounds (chunks)
VROW = H * (HD + 1)   # 780: V' row with ones col per head
BF16 = mybir.dt.bfloat16
F32R = mybir.dt.float32r
F32 = mybir.dt.float32


def q_rows(c):
    t = np.arange(32)
    u = np.arange(16)
    return (16 * (c + 8 * t)[:, None] + u[None, :]).reshape(-1)


def make_mask_ext(c):
    """mask_ext[kap, j, m]: for key block kb = 4r+j, q col (abs) 64r+m:
    valid iff 128j + kap <= 16c + 128*(m//16) + (m%16). r-independent."""
    kap = np.arange(128)[:, None, None]
    j = np.arange(4)[None, :, None]
    m = np.arange(192)[None, None, :]
    valid = (128 * j + kap) <= (16 * c + 128 * (m // 16) + (m % 16))
    return valid.astype(np.float32)


def fix_excess_waits(nc):
    """walrus rejects >1 sync wait per instruction; hoist extras onto NoOps."""
    k = 0
    for f in nc.m.functions:
        for bb in f.blocks:
            insts = bb.instructions
            i = 0
            while i < len(insts):
                ins = insts[i]
                si = getattr(ins, "sync_info", None)
                if si is not None and len(si.on_wait) > 1:
                    for w in si.on_wait[:-1]:
                        nop = mybir.InstNoOp(name=f"W-hoist-{k}", ins=[], outs=[])
                        k += 1
                        nop.engine = ins.engine
                        nop.sync_info = mybir.SyncInfo(on_wait=[w], on_update=[])
                        insts.insert(i, nop)
                        i += 1
                    ins.sync_info = mybir.SyncInfo(
                        on_wait=[si.on_wait[-1]], on_update=list(si.on_update))
                i += 1
    return k


def build(fix_waits=True, debug=False):
    nc = bass.Bass()
    xqt = nc.dram_tensor("xqt", [D, SQ], BF16, kind="ExternalInput")
    xt = nc.dram_tensor("xt", [D, T], BF16, kind="ExternalInput")
    wq = nc.dram_tensor("wq", [D, D], BF16, kind="ExternalInput")
    wk = nc.dram_tensor("wk", [D, D], BF16, kind="ExternalInput")
    wv = nc.dram_tensor("wv", [D, D], BF16, kind="ExternalInput")
    wo = nc.dram_tensor("wo", [D, D], BF16, kind="ExternalInput")
    bo = nc.dram_tensor("bo", [P, D], F32, kind="ExternalInput")
    maskx = nc.dram_tensor("maskx", [P, 4, 192], BF16, kind="ExternalInput")
    out = nc.dram_tensor("out", [SQ, D], F32, kind="ExternalOutput")
    if debug:
        dbg_qt = nc.dram_tensor("dbg_qt", [P, DC, SQ], BF16, kind="ExternalOutput")
        dbg_kt = nc.dram_tensor("dbg_kt", [P, DC, CH], BF16, kind="ExternalOutput")
        dbg_vt = nc.dram_tensor("dbg_vt", [P, 4, VROW], BF16, kind="ExternalOutput")
        dbg_pt = nc.dram_tensor("dbg_pt", [P, 2, 4, SQ], BF16, kind="ExternalOutput")
        dbg_ctxA = nc.dram_tensor("dbg_ctxA", [HD + 1, DC, SQ], F32, kind="ExternalOutput")
        dbg_ctxB = nc.dram_tensor("dbg_ctxB", [HD + 1, DC, SQ], F32, kind="ExternalOutput")
        dbg_ctxt = nc.dram_tensor("dbg_ctxt", [P, DC, SQ], BF16, kind="ExternalOutput")

    EXP = mybir.ActivationFunctionType.Exp

    with tile.TileContext(nc) as tc:
        with (
            tc.tile_pool(name="glob", bufs=1) as glob,
            tc.tile_pool(name="xc", bufs=2) as xcp,
            tc.tile_pool(name="ktp", bufs=2) as ktp,
            tc.tile_pool(name="vtp", bufs=2) as vtp,
            tc.tile_pool(name="att", bufs=2) as att,
            tc.tile_pool(name="nrm", bufs=2) as nrm,
            tc.tile_pool(name="ps_s", bufs=2, space="PSUM") as ps_s,
            tc.tile_pool(name="ps_c", bufs=2, space="PSUM") as ps_c,
        ):
            # ---- persistent tiles
            qt = glob.tile([P, DC, SQ], BF16)        # Q^T, head pair i on dc=i
            mask_sb = glob.tile([P, 4, 192], BF16)
            bo_bc = glob.tile([P, D], F32)
            wq_sb = glob.tile([P, DC, D], BF16)
            wk_sb = glob.tile([P, DC, D], BF16)
            wv_sb = glob.tile([P, DC, D], BF16)
            wo_sb = glob.tile([P, DC, D], BF16)
            xq_sb = glob.tile([P, DC, SQ], BF16)
            # unnormalized ctx + denominator row (row 64) per head, f32
            ctxA = glob.tile([HD + 1, DC, SQ], F32)  # even heads 2i
            ctxB = glob.tile([HD + 1, DC, SQ], F32)  # odd heads 2i+1
            ctxt = glob.tile([P, DC, SQ], BF16)      # normalized ctx^T for Wo
            o_sb = glob.tile([P, SQ // P, D], F32)
            ones64 = glob.tile([1, HD], F32R)
            rec_sc = glob.tile([1, H, SQ], F32R)     # per-head 1/den, 1 lane

            nc.sync.dma_start(wq_sb[:], wq.rearrange("(o p) d -> p o d", p=P))
            nc.sync.dma_start(xq_sb[:], xqt.rearrange("(o p) t -> p o t", p=P))
            nc.sync.dma_start(wk_sb[:], wk.rearrange("(o p) d -> p o d", p=P))
            nc.sync.dma_start(wv_sb[:], wv.rearrange("(o p) d -> p o d", p=P))
            nc.sync.dma_start(wo_sb[:], wo.rearrange("(o p) d -> p o d", p=P))
            nc.sync.dma_start(mask_sb[:], maskx[:])
            nc.sync.dma_start(bo_bc[:], bo[:])
            nc.vector.memset(ones64.bitcast(mybir.dt.uint32), 0x3F800000)

            xtv = xt.rearrange("(o p) t -> p o t", p=P)
            xts = [None] * NR
            kts = [None] * NR
            vts = [None] * NR
            for rr in range(2):
                xts[rr] = xcp.tile([P, DC, CH], BF16, tag="xc", name=f"xt{rr}")
                nc.sync.dma_start(xts[rr][:], xtv[:, :, rr * CH:(rr + 1) * CH])

            def proj_units(r):
                """Closures projecting chunk r's K^T and V' (7 units)."""
                units = []

                def k_unit(q2, r=r):
                    if q2 == 0:
                        kts[r] = ktp.tile([P, DC, CH], BF16, tag="kt", name=f"kt{r}")
                    kt = kts[r]
                    xtc = xts[r]
                    pp = ps_s.tile([P, 2, CH], F32, tag="s")
                    for par in range(2):
                        dc = 2 * q2 + par
                        for ko in range(DC):
                            nc.tensor.matmul(
                                pp[:, par, :], wk_sb[:, ko, dc * P:(dc + 1) * P],
                                xtc[:, ko, :], start=(ko == 0), stop=(ko == DC - 1))
                    nc.vector.tensor_copy(kt[:, 2 * q2:2 * q2 + 2, :], pp[:])

                def v_unit(tc4, r=r):
                    if tc4 == 0:
                        vts[r] = vtp.tile([P, 4, VROW], BF16, tag="vt", name=f"vt{r}")
                        v4i = vts[r].rearrange("p f (h c) -> p f h c", c=HD + 1)
                        nc.vector.memset(v4i[:, :, :, HD:HD + 1], 1.0)
                    vt = vts[r]
                    v4 = vt.rearrange("p f (h c) -> p f h c", c=HD + 1)
                    xtc = xts[r]
                    pp = ps_s.tile([P, 2, CH], F32, tag="s")
                    for nh in range(2):
                        for ko in range(DC):
                            nc.tensor.matmul(
                                pp[:, nh, 0:384], xtc[:, ko, tc4 * P:(tc4 + 1) * P],
                                wv_sb[:, ko, nh * 384:(nh + 1) * 384],
                                start=(ko == 0), stop=(ko == DC - 1))
                    nc.vector.tensor_copy(
                        v4[:, tc4, :, 0:HD].rearrange(
                            "p (n h) c -> p n h c", n=2),
                        pp[:, :, 0:384].rearrange("p n (h c) -> p n h c", c=HD))

                for q2 in range(DC // 2):
                    units.append(lambda q2=q2: k_unit(q2))
                for tc4 in range(4):
                    units.append(lambda tc4=tc4: v_unit(tc4))
                return units

            # ---- Q projection: qt[0:64, i] = head 2i, qt[64:128, i] = 2i+1
            for q2 in range(DC // 2):
                pp = ps_s.tile([P, 2, SQ], F32, tag="s")
                for par in range(2):
                    dc = 2 * q2 + par
                    for ko in range(DC):
                        nc.tensor.matmul(
                            pp[:, par, :], wq_sb[:, ko, dc * P:(dc + 1) * P],
                            xq_sb[:, ko, :], start=(ko == 0), stop=(ko == DC - 1))
                nc.vector.tensor_copy(qt[:, 2 * q2:2 * q2 + 2, :], pp[:])

            def o_unit(m):
                """Project finished t-block m through Wo and DMA it out."""
                op = ps_s.tile([P, 2, CH], F32, tag="s")
                for nh in range(2):
                    for dc in range(DC):
                        nc.tensor.matmul(
                            op[:, nh, 0:384], ctxt[:, dc, m * P:(m + 1) * P],
                            wo_sb[:, dc, nh * 384:(nh + 1) * 384],
                            start=(dc == 0), stop=(dc == DC - 1))
                nc.vector.tensor_add(
                    o_sb[:, m, :].rearrange("p (n c) -> p n c", n=2),
                    op[:, :, 0:384],
                    bo_bc.rearrange("p (n c) -> p n c", n=2))
                nc.sync.dma_start(
                    out.rearrange("(o p) d -> p o d", p=P)[:, m, :],
                    o_sb[:, m, :])

            # chunk 0 projection up front
            for u in proj_units(0):
                u()

            # ---- rounds: attend over chunk r; interleave projection of r+1
            for r in range(NR):
                scope = nc.named_scope(f"round{r}")
                scope.__enter__()
                N = SQ - 64 * r      # live q cols this round
                q0 = 64 * r
                if r + 2 < NR:
                    xts[r + 2] = xcp.tile([P, DC, CH], BF16, tag="xc",
                                        name=f"xt{r + 2}")
                    nc.sync.dma_start(
                        xts[r + 2][:], xtv[:, :, (r + 2) * CH:(r + 3) * CH])
                kt, vt = kts[r], vts[r]
                if debug and r == 0:
                    nc.sync.dma_start(dbg_kt[:], kt[:])
                    nc.sync.dma_start(dbg_vt[:], vt[:])
                nxt = proj_units(r + 1) if r + 1 < NR else []

                if N > 256:
                    packs = [[0], [1], [2], [3]]
                elif N > 128:
                    packs = [[0, 1], [2, 3]]
                else:
                    packs = [[0, 1, 2, 3]]
                M = min(192, N)

                # flat pack pipeline: S/exp stage runs one pack ahead of
                # the PV stage so the exp stream never starves at pair
                # boundaries; proj fill units slot between them.
                items = []
                for i in range(DC):
                    for pidx, pk in enumerate(packs):
                        items.append(
                            (i, pk, pidx == 0, pidx == len(packs) - 1))
                n = len(items)
                pts = {}
                cpss = {}

                def emit_s(idx):
                    i, pk, first, _ = items[idx]
                    if first:
                        pts[i] = att.tile([P, 2, 4, SQ], BF16, tag="pt",
                                          name=f"pt{r}_{i}")
                    pt = pts[i]
                    sps = ps_s.tile([P, 2, CH], F32, tag="s")
                    for idx2, j in enumerate(pk):
                        for hh in range(2):
                            nc.tensor.matmul(
                                sps[0:P, hh, idx2 * N:(idx2 + 1) * N],
                                kt[64 * hh:64 * hh + 64, i, j * P:(j + 1) * P],
                                qt[64 * hh:64 * hh + 64, i, q0:SQ],
                                start=True, stop=True)
                    W = len(pk) * N
                    j0 = pk[0]
                    nc.scalar.activation(
                        pt[:, :, j0:j0 + len(pk), 0:N], sps[:, :, 0:W],
                        EXP, scale=0.125)
                    for hh, eng in ((0, nc.vector), (1, nc.gpsimd)):
                        eng.tensor_mul(
                            pt[:, hh, j0:j0 + len(pk), 0:M],
                            pt[:, hh, j0:j0 + len(pk), 0:M],
                            mask_sb[:, j0:j0 + len(pk), 0:M])

                def emit_pv(idx):
                    i, pk, first, last = items[idx]
                    if first:
                        cpss[i] = ps_c.tile([P, 2, SQ], F32, tag="c",
                                            name=f"cps{r}_{i}")
                    cps = cpss[i]
                    for j in pk:
                        for hh in range(2):
                            h = 2 * i + hh
                            nc.tensor.matmul(
                                cps[0:HD + 1, hh, 0:N],
                                vt[:, j, h * (HD + 1):(h + 1) * (HD + 1)],
                                pts[i][:, hh, j, 0:N],
                                start=(j == 0), stop=(j == 3),
                                skip_group_check=True)
                    if not last:
                        return
                    if debug and r == 0 and i == 0:
                        nc.sync.dma_start(dbg_pt[:], pts[i][:])
                    for hh, ctx in ((0, ctxA), (1, ctxB)):
                        if r == 0:
                            nc.vector.tensor_copy(
                                ctx[:, i, :], cps[0:HD + 1, hh, :])
                        else:
                            nc.vector.tensor_add(
                                ctx[:, i, q0:SQ], ctx[:, i, q0:SQ],
                                cps[0:HD + 1, hh, 0:N])

                units_done = 0
                emit_s(0)
                for k in range(n):
                    if k + 1 < n:
                        emit_s(k + 1)
                    owed = ((k + 1) * len(nxt)) // n
                    if units_done < owed and units_done < len(nxt):
                        nxt[units_done]()
                        units_done += 1
                    emit_pv(k)
                for u in nxt[units_done:]:
                    u()
                scope.__exit__(None, None, None)

            # ---- tail: normalize all heads, then project + emit blocks
            recg = nrm.tile([H, SQ], F32, tag="rg")
            rec12 = nrm.tile([H, SQ], F32R, tag="r12")
            nc.sync.dma_start(recg[0:DC, :], ctxA[HD:HD + 1, :, :])
            nc.sync.dma_start(recg[DC:H, :], ctxB[HD:HD + 1, :, :])
            with nc.allow_low_precision(reason="f32r broadcast"):
                nc.vector.reciprocal(rec12[:], recg[:])
            nc.sync.dma_start(rec_sc[:], rec12[:])
            for i in range(DC):
                for hh, ctx in ((0, ctxA), (1, ctxB)):
                    bc = ps_c.tile([HD, 2, SQ], F32, tag="c")
                    nc.tensor.matmul(
                        bc[:, 0, :], ones64[:], rec_sc[0:1, i + DC * hh, :],
                        start=True, stop=True)
                    nc.vector.tensor_mul(
                        ctxt[64 * hh:64 * hh + 64, i, :],
                        ctx[0:HD, i, :], bc[:, 0, :])
            for m in range(SQ // P):
                o_unit(m)

            if debug:
                nc.sync.dma_start(dbg_qt[:], qt[:])
                nc.sync.dma_start(dbg_ctxA[:], ctxA[:])
                nc.sync.dma_start(dbg_ctxB[:], ctxB[:])

            if debug:
                nc.sync.dma_start(dbg_ctxt[:], ctxt[:])

    if fix_waits:
        fix_excess_waits(nc)
    return nc


_NC_CACHE = None


def _get_nc():
    global _NC_CACHE
    if _NC_CACHE is None:
        _NC_CACHE = build()
    return _NC_CACHE


def _run(inputs, trace=False):
    import ml_dtypes
    bf16 = ml_dtypes.bfloat16

    x = np.asarray(inputs["x"], dtype=np.float32)
    Wq = np.asarray(inputs["Wq"], dtype=np.float32).astype(bf16)
    Wk = np.asarray(inputs["Wk"], dtype=np.float32).astype(bf16)
    Wv = np.asarray(inputs["Wv"], dtype=np.float32).astype(bf16)
    Wo = np.asarray(inputs["Wo"], dtype=np.float32).astype(bf16)
    bo_v = np.ascontiguousarray(
        np.broadcast_to(np.asarray(inputs["bo"], dtype=np.float32).reshape(1, D),
                        (P, D)))
    xf = x.reshape(T, D)
    xt_full = np.ascontiguousarray(xf.T).astype(bf16)

    nc_prog = _get_nc()
    in_maps = []
    for c in range(NC):
        rows = q_rows(c)
        in_maps.append({
            "xqt": np.ascontiguousarray(xf[rows].T).astype(bf16),
            "xt": xt_full,
            "wq": Wq, "wk": Wk, "wv": Wv, "wo": Wo, "bo": bo_v,
            "maskx": make_mask_ext(c).astype(bf16),
        })
    res = run_bass_kernel_spmd(
        nc_prog, in_maps, core_ids=list(range(NC)), trace=trace)
    full = np.empty((T, D), dtype=np.float32)
    for c in range(NC):
        full[q_rows(c)] = res.results[c]["out"]
    return full.reshape(1, T, D), res


def kernel(**inputs) -> np.ndarray:
    out, _ = _run(inputs, trace=False)
    return out


# revision 24
# speedup vs baseline: 1.2505x; 1.0042x over previous
"""Causal MHA (B=1, T=4096, D=768, H=12) on 8 TRN2 NeuronCores.

Strategy (v2)
-------------
- Sequence-parallel over T with row-interleaved q-assignment so every core
  runs the *same* program on identically-shaped causal work:
  core c owns q rows {16*(c+8*t)+u}, i.e. 16-row miniblocks strided by 8.
- No collectives (bass collectives run at ~50-60 GB/s with ~10us/step
  latency floors -- an all-gather of K/V would cost 200us+). Each core
  redundantly projects the FULL K^T and V' from the full x, one 512-key
  chunk at a time, software-pipelined with attention over the previous
  chunk. K^T/V' chunks live only in SBUF (bf16) -- no DRAM roundtrip.
- All matmuls in bf16 (PSUM accumulates f32). Scores ~N(0,1), so softmax
  skips the running max: exp(s/8) never overflows. Unnormalized context +
  denominator (ones-column on V') accumulate in SBUF f32 across chunks.
- S^T per head pair runs as two concurrent K=64 row-tiled matmuls
  (tile_position (0,0)/(64,0) auto-derived from base partitions).
- Per-round causal trim: round r covers q cols [64r, 512), one
  r-independent 192-col mask zeroes the sub-diagonal prefix + diagonal.
- exp batched per (pair, kb) across both heads' PSUM banks, with bank
  packing for the small-N rounds to amortize ACT instruction overhead.
"""
import sys

sys.path.insert(0, "/opt/trn_rl_repo")

import numpy as np

import concourse.bass as bass
import concourse.mybir as mybir
import concourse.tile as tile
from concourse.bass_utils import run_bass_kernel_spmd

P = 128
T, D, H, HD = 4096, 768, 12, 64
NC = 8
SQ = T // NC          # 512 q rows per core
CH = 512              # kv chunk (4 key blocks of 128)
DC = D // P           # 6 contraction chunks
NR = 8                # rounds (chunks)
VROW = H * (HD + 1)   # 780: V' row with ones col per head
BF16 = mybir.dt.bfloat16
F32R = mybir.dt.float32r
F32 = mybir.dt.float32


def q_rows(c):
    t = np.arange(32)
    u = np.arange(16)
    return (16 * (c + 8 * t)[:, None] + u[None, :]).reshape(-1)


def make_mask_ext(c):
    """mask_ext[kap, j, m]: for key block kb = 4r+j, q col (abs) 64r+m:
    valid iff 128j + kap <= 16c + 128*(m//16) + (m%16). r-independent."""
    kap = np.arange(128)[:, None, None]
    j = np.arange(4)[None, :, None]
    m = np.arange(192)[None, None, :]
    valid = (128 * j + kap) <= (16 * c + 128 * (m // 16) + (m % 16))
    return valid.astype(np.float32)


def fix_excess_waits(nc):
    """walrus rejects >1 sync wait per instruction; hoist extras onto NoOps."""
    k = 0
    for f in nc.m.functions:
        for bb in f.blocks:
            insts = bb.instructions
            i = 0
            while i < len(insts):
                ins = insts[i]
                si = getattr(ins, "sync_info", None)
                if si is not None and len(si.on_wait) > 1:
                    for w in si.on_wait[:-1]:
                        nop = mybir.InstNoOp(name=f"W-hoist-{k}", ins=[], outs=[])
                        k += 1
                        nop.engine = ins.engine
                        nop.sync_info = mybir.SyncInfo(on_wait=[w], on_update=[])
                        insts.insert(i, nop)
                        i += 1
                    ins.sync_info = mybir.SyncInfo(
                        on_wait=[si.on_wait[-1]], on_update=list(si.on_update))
                i += 1
    return k


def build(fix_waits=True, debug=False):
    nc = bass.Bass()
    xqt = nc.dram_tensor("xqt", [D, SQ], BF16, kind="ExternalInput")
    xt = nc.dram_tensor("xt", [D, T], BF16, kind="ExternalInput")
    wq = nc.dram_tensor("wq", [D, D], BF16, kind="ExternalInput")
    wk = nc.dram_tensor("wk", [D, D], BF16, kind="ExternalInput")
    wv = nc.dram_tensor("wv", [D, D], BF16, kind="ExternalInput")
    wo = nc.dram_tensor("wo", [D, D], BF16, kind="ExternalInput")
    bo = nc.dram_tensor("bo", [P, D], F32, kind="ExternalInput")
    maskx = nc.dram_tensor("maskx", [P, 4, 192], BF16, kind="ExternalInput")
    out = nc.dram_tensor("out", [SQ, D], F32, kind="ExternalOutput")
    if debug:
        dbg_qt = nc.dram_tensor("dbg_qt", [P, DC, SQ], BF16, kind="ExternalOutput")
        dbg_kt = nc.dram_tensor("dbg_kt", [P, DC, CH], BF16, kind="ExternalOutput")
        dbg_vt = nc.dram_tensor("dbg_vt", [P, 4, VROW], BF16, kind="ExternalOutput")
        dbg_pt = nc.dram_tensor("dbg_pt", [P, 2, 4, SQ], BF16, kind="ExternalOutput")
        dbg_ctxA = nc.dram_tensor("dbg_ctxA", [HD + 1, DC, SQ], F32, kind="ExternalOutput")
        dbg_ctxB = nc.dram_tensor("dbg_ctxB", [HD + 1, DC, SQ], F32, kind="ExternalOutput")
        dbg_ctxt = nc.dram_tensor("dbg_ctxt", [P, DC, SQ], BF16, kind="ExternalOutput")

    EXP = mybir.ActivationFunctionType.Exp

    with tile.TileContext(nc) as tc:
        with (
            tc.tile_pool(name="glob", bufs=1) as glob,
            tc.tile_pool(name="xc", bufs=2) as xcp,
            tc.tile_pool(name="ktp", bufs=2) as ktp,
            tc.tile_pool(name="vtp", bufs=2) as vtp,
            tc.tile_pool(name="att", bufs=3) as att,
            tc.tile_pool(name="nrm", bufs=2) as nrm,
            tc.tile_pool(name="ps_s", bufs=2, space="PSUM") as ps_s,
            tc.tile_pool(name="ps_c", bufs=2, space="PSUM") as ps_c,
        ):
            # ---- persistent tiles
            qt = glob.tile([P, DC, SQ], BF16)        # Q^T, head pair i on dc=i
            mask_sb = glob.tile([P, 4, 192], BF16)
            bo_bc = glob.tile([P, D], F32)
            wq_sb = glob.tile([P, DC, D], BF16)
            wk_sb = glob.tile([P, DC, D], BF16)
            wv_sb = glob.tile([P, DC, D], BF16)
            wo_sb = glob.tile([P, DC, D], BF16)
            xq_sb = glob.tile([P, DC, SQ], BF16)
            # unnormalized ctx + denominator row (row 64) per head, f32
            ctxA = glob.tile([HD + 1, DC, SQ], F32)  # even heads 2i
            ctxB = glob.tile([HD + 1, DC, SQ], F32)  # odd heads 2i+1
            ctxt = glob.tile([P, DC, SQ], BF16)      # normalized ctx^T for Wo
            o_sb = glob.tile([P, SQ // P, D], F32)
            ones64 = glob.tile([1, HD], F32R)
            rec_sc = glob.tile([1, H, SQ], F32R)     # per-head 1/den, 1 lane

            nc.sync.dma_start(wq_sb[:], wq.rearrange("(o p) d -> p o d", p=P))
            nc.sync.dma_start(xq_sb[:], xqt.rearrange("(o p) t -> p o t", p=P))
            nc.sync.dma_start(wk_sb[:], wk.rearrange("(o p) d -> p o d", p=P))
            nc.sync.dma_start(wv_sb[:], wv.rearrange("(o p) d -> p o d", p=P))
            nc.sync.dma_start(wo_sb[:], wo.rearrange("(o p) d -> p o d", p=P))
            nc.sync.dma_start(mask_sb[:], maskx[:])
            nc.sync.dma_start(bo_bc[:], bo[:])
            nc.vector.memset(ones64.bitcast(mybir.dt.uint32), 0x3F800000)

            xtv = xt.rearrange("(o p) t -> p o t", p=P)
            xts = [None] * NR
            kts = [None] * NR
            vts = [None] * NR
            for rr in range(2):
                xts[rr] = xcp.tile([P, DC, CH], BF16, tag="xc", name=f"xt{rr}")
                nc.sync.dma_start(xts[rr][:], xtv[:, :, rr * CH:(rr + 1) * CH])

            def proj_units(r):
                """Closures projecting chunk r's K^T and V' (7 units)."""
                units = []

                def k_unit(q2, r=r):
                    if q2 == 0:
                        kts[r] = ktp.tile([P, DC, CH], BF16, tag="kt", name=f"kt{r}")
                    kt = kts[r]
                    xtc = xts[r]
                    pp = ps_s.tile([P, 2, CH], F32, tag="s")
                    for par in range(2):
                        dc = 2 * q2 + par
                        for ko in range(DC):
                            nc.tensor.matmul(
                                pp[:, par, :], wk_sb[:, ko, dc * P:(dc + 1) * P],
                                xtc[:, ko, :], start=(ko == 0), stop=(ko == DC - 1))
                    nc.vector.tensor_copy(kt[:, 2 * q2:2 * q2 + 2, :], pp[:])

                def v_unit(tc4, r=r):
                    if tc4 == 0:
                        vts[r] = vtp.tile([P, 4, VROW], BF16, tag="vt", name=f"vt{r}")
                        v4i = vts[r].rearrange("p f (h c) -> p f h c", c=HD + 1)
                        nc.vector.memset(v4i[:, :, :, HD:HD + 1], 1.0)
                    vt = vts[r]
                    v4 = vt.rearrange("p f (h c) -> p f h c", c=HD + 1)
                    xtc = xts[r]
                    pp = ps_s.tile([P, 2, CH], F32, tag="s")
                    for nh in range(2):
                        for ko in range(DC):
                            nc.tensor.matmul(
                                pp[:, nh, 0:384], xtc[:, ko, tc4 * P:(tc4 + 1) * P],
                                wv_sb[:, ko, nh * 384:(nh + 1) * 384],
                                start=(ko == 0), stop=(ko == DC - 1))
                    nc.vector.tensor_copy(
                        v4[:, tc4, :, 0:HD].rearrange(
                            "p (n h) c -> p n h c", n=2),
                        pp[:, :, 0:384].rearrange("p n (h c) -> p n h c", c=HD))

                for q2 in range(DC // 2):
                    units.append(lambda q2=q2: k_unit(q2))
                for tc4 in range(4):
                    units.append(lambda tc4=tc4: v_unit(tc4))
                return units

            # ---- Q projection: qt[0:64, i] = head 2i, qt[64:128, i] = 2i+1
            for q2 in range(DC // 2):
                pp = ps_s.tile([P, 2, SQ], F32, tag="s")
                for par in range(2):
                    dc = 2 * q2 + par
                    for ko in range(DC):
                        nc.tensor.matmul(
                            pp[:, par, :], wq_sb[:, ko, dc * P:(dc + 1) * P],
                            xq_sb[:, ko, :], start=(ko == 0), stop=(ko == DC - 1))
                nc.vector.tensor_copy(qt[:, 2 * q2:2 * q2 + 2, :], pp[:])

            def o_unit(m):
                """Project finished t-block m through Wo and DMA it out."""
                op = ps_s.tile([P, 2, CH], F32, tag="s")
                for nh in range(2):
                    for dc in range(DC):
                        nc.tensor.matmul(
                            op[:, nh, 0:384], ctxt[:, dc, m * P:(m + 1) * P],
                            wo_sb[:, dc, nh * 384:(nh + 1) * 384],
                            start=(dc == 0), stop=(dc == DC - 1))
                nc.vector.tensor_add(
                    o_sb[:, m, :].rearrange("p (n c) -> p n c", n=2),
                    op[:, :, 0:384],
                    bo_bc.rearrange("p (n c) -> p n c", n=2))
                nc.sync.dma_start(
                    out.rearrange("(o p) d -> p o d", p=P)[:, m, :],
                    o_sb[:, m, :])

            # chunk 0 projection up front
            for u in proj_units(0):
                u()

            # ---- rounds: attend over chunk r; interleave projection of r+1
            for r in range(NR):
                scope = nc.named_scope(f"round{r}")
                scope.__enter__()
                N = SQ - 64 * r      # live q cols this round
                q0 = 64 * r
                if r + 2 < NR:
                    xts[r + 2] = xcp.tile([P, DC, CH], BF16, tag="xc",
                                        name=f"xt{r + 2}")
                    nc.sync.dma_start(
                        xts[r + 2][:], xtv[:, :, (r + 2) * CH:(r + 3) * CH])
                kt, vt = kts[r], vts[r]
                if debug and r == 0:
                    nc.sync.dma_start(dbg_kt[:], kt[:])
                    nc.sync.dma_start(dbg_vt[:], vt[:])
                nxt = proj_units(r + 1) if r + 1 < NR else []

                if N > 256:
                    packs = [[0], [1], [2], [3]]
                elif N > 128:
                    packs = [[0, 1], [2, 3]]
                else:
                    packs = [[0, 1, 2, 3]]
                M = min(192, N)

                # flat pack pipeline: S/exp stage runs one pack ahead of
                # the PV stage so the exp stream never starves at pair
                # boundaries; proj fill units slot between them.
                items = []
                for i in range(DC):
                    for pidx, pk in enumerate(packs):
                        items.append(
                            (i, pk, pidx == 0, pidx == len(packs) - 1))
                n = len(items)
                pts = {}
                cpss = {}

                def emit_s(idx):
                    i, pk, first, _ = items[idx]
                    if first:
                        pts[i] = att.tile([P, 2, 4, SQ], BF16, tag="pt",
                                          name=f"pt{r}_{i}")
                    pt = pts[i]
                    sps = ps_s.tile([P, 2, CH], F32, tag="s")
                    for idx2, j in enumerate(pk):
                        for hh in range(2):
                            nc.tensor.matmul(
                                sps[0:P, hh, idx2 * N:(idx2 + 1) * N],
                                kt[64 * hh:64 * hh + 64, i, j * P:(j + 1) * P],
                                qt[64 * hh:64 * hh + 64, i, q0:SQ],
                                start=True, stop=True)
                    W = len(pk) * N
                    j0 = pk[0]
                    nc.scalar.activation(
                        pt[:, :, j0:j0 + len(pk), 0:N], sps[:, :, 0:W],
                        EXP, scale=0.125)
                    for hh, eng in ((0, nc.vector), (1, nc.gpsimd)):
                        eng.tensor_mul(
                            pt[:, hh, j0:j0 + len(pk), 0:M],
                            pt[:, hh, j0:j0 + len(pk), 0:M],
                            mask_sb[:, j0:j0 + len(pk), 0:M])

                def emit_pv(idx):
                    i, pk, first, last = items[idx]
                    if first:
                        cpss[i] = ps_c.tile([P, 2, SQ], F32, tag="c",
                                            name=f"cps{r}_{i}")
                    cps = cpss[i]
                    for j in pk:
                        for hh in range(2):
                            h = 2 * i + hh
                            nc.tensor.matmul(
                                cps[0:HD + 1, hh, 0:N],
                                vt[:, j, h * (HD + 1):(h + 1) * (HD + 1)],
                                pts[i][:, hh, j, 0:N],
                                start=(j == 0), stop=(j == 3),
                                skip_group_check=True)
                    if not last:
                        return
                    if debug and r == 0 and i == 0:
                        nc.sync.dma_start(dbg_pt[:], pts[i][:])
                    for hh, ctx in ((0, ctxA), (1, ctxB)):
                        if r == 0:
                            nc.vector.tensor_copy(
                                ctx[:, i, :], cps[0:HD + 1, hh, :])
                        else:
                            nc.vector.tensor_add(
                                ctx[:, i, q0:SQ], ctx[:, i, q0:SQ],
                                cps[0:HD + 1, hh, 0:N])

                units_done = 0
                emit_s(0)
                if n > 1:
                    emit_s(1)
                for k in range(n):
                    if k + 2 < n:
                        emit_s(k + 2)
                    owed = ((k + 1) * len(nxt)) // n
                    if units_done < owed and units_done < len(nxt):
                        nxt[units_done]()
                        units_done += 1
                    emit_pv(k)
                for u in nxt[units_done:]:
                    u()
                scope.__exit__(None, None, None)

            # ---- tail: normalize all heads, then project + emit blocks
            recg = nrm.tile([H, SQ], F32, tag="rg")
            rec12 = nrm.tile([H, SQ], F32R, tag="r12")
            nc.sync.dma_start(recg[0:DC, :], ctxA[HD:HD + 1, :, :])
            nc.sync.dma_start(recg[DC:H, :], ctxB[HD:HD + 1, :, :])
            with nc.allow_low_precision(reason="f32r broadcast"):
                nc.vector.reciprocal(rec12[:], recg[:])
            nc.sync.dma_start(rec_sc[:], rec12[:])
            for i in range(DC):
                for hh, ctx in ((0, ctxA), (1, ctxB)):
                    bc = ps_c.tile([HD, 2, SQ], F32, tag="c")
                    nc.tensor.matmul(
                        bc[:, 0, :], ones64[:], rec_sc[0:1, i + DC * hh, :],
                        start=True, stop=True)
                    nc.vector.tensor_mul(
                        ctxt[64 * hh:64 * hh + 64, i, :],
                        ctx[0:HD, i, :], bc[:, 0, :])
            for m in range(SQ // P):
                o_unit(m)

            if debug:
                nc.sync.dma_start(dbg_qt[:], qt[:])
                nc.sync.dma_start(dbg_ctxA[:], ctxA[:])
                nc.sync.dma_start(dbg_ctxB[:], ctxB[:])

            if debug:
                nc.sync.dma_start(dbg_ctxt[:], ctxt[:])

    if fix_waits:
        fix_excess_waits(nc)
    return nc


_NC_CACHE = None


def _get_nc():
    global _NC_CACHE
    if _NC_CACHE is None:
        _NC_CACHE = build()
    return _NC_CACHE


def _run(inputs, trace=False):
    import ml_dtypes
    bf16 = ml_dtypes.bfloat16

    x = np.asarray(inputs["x"], dtype=np.float32)
    Wq = np.asarray(inputs["Wq"], dtype=np.float32).astype(bf16)
    Wk = np.asarray(inputs["Wk"], dtype=np.float32).astype(bf16)
    Wv = np.asarray(inputs["Wv"], dtype=np.float32).astype(bf16)
    Wo = np.asarray(inputs["Wo"], dtype=np.float32).astype(bf16)
    bo_v = np.ascontiguousarray(
        np.broadcast_to(np.asarray(inputs["bo"], dtype=np.float32).reshape(1, D),
                        (P, D)))
    xf = x.reshape(T, D)
    xt_full = np.ascontiguousarray(xf.T).astype(bf16)

    nc_prog = _get_nc()
    in_maps = []
    for c in range(NC):
        rows = q_rows(c)
        in_maps.append({
            "xqt": np.ascontiguousarray(xf[rows].T).astype(bf16),
            "xt": xt_full,
            "wq": Wq, "wk": Wk, "wv": Wv, "wo": Wo, "bo": bo_v,
            "maskx": make_mask_ext(c).astype(bf16),
        })
    res = run_bass_kernel_spmd(
        nc_prog, in_maps, core_ids=list(range(NC)), trace=trace)
    full = np.empty((T, D), dtype=np.float32)
    for c in range(NC):
        full[q_rows(c)] = res.results[c]["out"]
    return full.reshape(1, T, D), res


def kernel(**inputs) -> np.ndarray:
    out, _ = _run(inputs, trace=False)
    return out
